# revision 1
# baseline (speedup 1.0000x reference)
"""Multi-head self-attention (B=1, S=4096, DIM=768, H=12) on 8 Trainium2
NeuronCores.

Sharding: tensor-parallel over heads. Core c computes
  - full attention for head hA = c            (heads 0..7, all 4096 queries)
  - half attention for head hB = 8 + c//2     (heads 8..11, query half c%2)
Each core computes its own K/V projections for its two heads (from x^T
streamed through SBUF once), runs flash-style attention entirely on-chip
(scores never touch HBM), applies its heads' slice of the output projection,
and returns the transposed partial projection. The host sums the per-core
partial projections (the tensor-parallel all-reduce), adds b_proj, and
transposes back.

On-chip layout is fully transposed (feature dims on partitions). Job A lives
on partitions 0:64, job B on 64:128, so the two jobs' QK^T matmuls execute
concurrently in disjoint PE row-groups (explicit tile_position). The QKV
projections are packed pairwise ([wqA|wkB], [wkA|wvB]) so each matmul fills
the full 128-wide array and lands each result on the partition range its
consumer needs. PV uses a stationary [V | 1] block so the softmax denominator
falls out as row 64 of the accumulator; no max-subtraction (scores are O(5)
for this problem's N(0,1)-scale inputs, exp cannot overflow fp32).

Matmuls run as float32r (full-rate fp32, ~1e-4 relative error).
"""

import numpy as np

DIM = 768
HEADS = 12
HD = 64
SCALE = HD ** (-0.5)
S = 4096
SH = 2048  # seq half
NCORES = 8
KT = DIM // 128  # 6 k-tiles over the 768 contraction dim
NKT = S // 128   # 32 k-tiles over the 4096 sequence dim

_CACHE: dict = {}
SIM_STATIC = False  # profile scripts set True: TimelineSim can't eval branches


def _build_nc():
    import concourse.bacc as bacc
    import concourse.tile as tile
    from concourse import mybir
    from concourse.masks import make_identity

    f32 = mybir.dt.float32
    f32r = mybir.dt.float32r
    bf16 = mybir.dt.bfloat16
    EXP = mybir.ActivationFunctionType.Exp

    nc = bacc.Bacc("TRN2", target_bir_lowering=False)

    # ---- DRAM I/O (per-core) ----
    xT = nc.dram_tensor("xT", [DIM, S], f32r, kind="ExternalInput")
    # packed: [:,0:128]=[wqA*s|wkB], [:,128:256]=[wkA|wvB], [:,256:384]=[wvA|wqB*s]
    wall = nc.dram_tensor("wall", [DIM, 384], f32r, kind="ExternalInput")
    # cols: b1=[bqA*s;bkB], b2=[bkA;bvB], b3=[bvA;bqB*s]
    ball = nc.dram_tensor("ball", [128, 4], f32, kind="ExternalInput")
    wpall = nc.dram_tensor("wpall", [HD, 2 * DIM], f32r, kind="ExternalInput")
    yA = nc.dram_tensor("yA", [DIM, S], f32, kind="ExternalOutput")
    yB = nc.dram_tensor("yB", [DIM, SH], f32, kind="ExternalOutput")

    with tile.TileContext(nc) as tc:
        _emit(nc, tc, mybir, make_identity, f32, f32r, bf16, EXP,
              xT, wall, ball, wpall, yA, yB)

    nc.compile()
    return nc


def _emit(nc, tc, mybir, make_identity, f32, f32r, bf16, EXP,
          xT, wall, ball, wpall, yA, yB):
    with tc.tile_pool(name="consts", bufs=1) as consts, \
         tc.tile_pool(name="persist", bufs=1) as persist, \
         tc.tile_pool(name="xcp", bufs=2) as xcp, \
         tc.tile_pool(name="vtw", bufs=2) as vtw, \
         tc.tile_pool(name="ptp", bufs=4) as ptp, \
         tc.tile_pool(name="ocp", bufs=4) as ocp, \
         tc.tile_pool(name="normp", bufs=3) as normp, \
         tc.tile_pool(name="atp", bufs=4) as atp, \
         tc.tile_pool(name="outp", bufs=3) as outp, \
         tc.tile_pool(name="ps_gp", bufs=2, space="PSUM") as ps_gp, \
         tc.tile_pool(name="ps_st", bufs=2, space="PSUM") as ps_st, \
         tc.tile_pool(name="ps_o", bufs=1, space="PSUM") as ps_o:

        # ---- constants & weights ----
        ident = consts.tile([128, 128], f32, tag="ident", name="ident")
        make_identity(nc, ident[:])
        ones32 = consts.tile([128, NKT], f32, tag="ones32", name="ones32")
        nc.vector.memset(ones32[:], 1.0)

        wall_t = []
        for k in range(KT):
            t = consts.tile([128, 384], f32r, tag=f"wall{k}", name=f"wall{k}")
            nc.sync.dma_start(out=t[:], in_=wall[k * 128:(k + 1) * 128, :])
            wall_t.append(t)
        w1_t = [t[:, 0:128] for t in wall_t]
        w2_t = [t[:, 128:256] for t in wall_t]
        w3_t = [t[:, 256:384] for t in wall_t]
        ballt = consts.tile([128, 4], f32, tag="ball", name="ball")
        nc.sync.dma_start(out=ballt[:], in_=ball[:])
        bias = {f"b{i + 1}": ballt[:, i:i + 1] for i in range(4)}
        wpt = consts.tile([HD, 2 * DIM], f32r, tag="wpall", name="wpall")
        nc.sync.dma_start(out=wpt[:], in_=wpall[:])
        wp_t = {"A": wpt[:, 0:DIM], "B": wpt[:, DIM:2 * DIM]}

        # ---- persistent on-chip tensors ----
        # QAB rows 0:64 = Q^T(A) over all 4096 q; rows 64:128 = Q^T(B) cols 0:2048
        QAB = persist.tile([128, S], f32r, tag="QAB", name="QAB")
        # KAB rows 0:64 = K^T(A); rows 64:128 = K^T(B)
        KAB = persist.tile([128, S], f32r, tag="KAB", name="KAB")
        # V_t[:, kt, 0:64]=V_A, col 64=ones, 65:129=V_B, col 129=ones
        V_t = persist.tile([128, NKT, 130], f32r, tag="V", name="V")
        nc.vector.tensor_copy(V_t[:, :, HD], ones32[:])
        nc.vector.tensor_copy(V_t[:, :, 65 + HD], ones32[:])

        # ---- QKV projections: x^T streamed once in [128,1024] chunks ----
        xTr = xT.rearrange("(k p) q -> p k q", p=128)

        def main_chunk(qc):  # qc in 0..3, covers q-cols 1024*qc..+1024
            xc = xcp.tile([128, KT, 1024], f32r, tag="xc", name="xc")
            # halves: consumers of the first 512 columns unblock earlier
            o = qc * 1024
            nc.sync.dma_start(out=xc[:, :, 0:512], in_=xTr[:, :, o:o + 512])
            nc.sync.dma_start(out=xc[:, :, 512:1024],
                              in_=xTr[:, :, o + 512:o + 1024])
            for h in range(2):
                cs = slice(qc * 1024 + h * 512, qc * 1024 + (h + 1) * 512)
                hs = slice(h * 512, (h + 1) * 512)
                ps1 = ps_gp.tile([128, 512], f32, tag="gp", name="gp1")
                for k in range(KT):
                    nc.tensor.matmul(ps1[:], w1_t[k], xc[:, k, hs],
                                     start=(k == 0), stop=(k == KT - 1))
                nc.vector.tensor_scalar_add(QAB[0:HD, cs], ps1[0:HD, :],
                                            bias["b1"][0:HD, :])
                nc.vector.tensor_scalar_add(KAB[HD:128, cs], ps1[HD:128, :],
                                            bias["b1"][HD:128, :])
                ps2 = ps_gp.tile([128, 512], f32, tag="gp", name="gp2")
                for k in range(KT):
                    nc.tensor.matmul(ps2[:], w2_t[k], xc[:, k, hs],
                                     start=(k == 0), stop=(k == KT - 1))
                vt = vtw.tile([128, 512], f32, tag="vt", name="vt")
                nc.vector.tensor_scalar_add(KAB[0:HD, cs], ps2[0:HD, :],
                                            bias["b2"][0:HD, :])
                nc.vector.tensor_scalar_add(vt[HD:128, :], ps2[HD:128, :],
                                            bias["b2"][HD:128, :])
                ps3 = ps_gp.tile([128, 512], f32, tag="gp", name="gp3")
                for k in range(KT):
                    nc.tensor.matmul(ps3[:], w3_t[k], xc[:, k, hs],
                                     start=(k == 0), stop=(k == KT - 1))
                nc.vector.tensor_scalar_add(vt[0:HD, :], ps3[0:HD, :],
                                            bias["b3"][0:HD, :])
                nc.vector.tensor_scalar_add(QAB[HD:128, cs], ps3[HD:128, :],
                                            bias["b3"][HD:128, :])
                tp = ps_gp.tile([128, 512], f32, tag="gp", name="gp4")
                for t4 in range(4):
                    kt = qc * 8 + h * 4 + t4
                    ts_ = slice(t4 * 128, (t4 + 1) * 128)
                    nc.tensor.transpose(tp[:, ts_], vt[:, ts_], ident[:])
                    nc.vector.tensor_copy(V_t[:, kt, 0:HD],
                                          tp[:, t4 * 128:t4 * 128 + HD])
                    nc.vector.tensor_copy(V_t[:, kt, 65:65 + HD],
                                          tp[:, t4 * 128 + HD:t4 * 128 + 128])

        # ---- attention super-sweeps + fused projection ----
        # each super-sweep handles two 512-query jobs (j0,q0) and (j1,q1)
        # (q in units of 512 columns of that job's Q rows).
        # Sweep 0 is interleaved with the QKV chunk stream (its kt range only
        # needs x chunks <= kt//8), each PV pair is deferred one kt so the
        # next S^T never queues behind it, and each sweep's normalize+proj is
        # deferred into the next sweep's slack.
        import concourse.bass as bass
        # relocate job B's q-columns (core-parity dependent) to a fixed tile
        pid = nc.sync.partition_id()
        QBloc = persist.tile([128, SH], f32r, tag="QBloc", name="QBloc")
        vo = {"A": 0, "B": 65}
        rowsl = {"A": slice(0, HD), "B": slice(HD, 128)}
        tpos = {"A": (0, 0), "B": (64, 0)}
        sweeps = [("A", 0, "A", 1), ("A", 2, "B", 0), ("A", 3, "B", 1),
                  ("A", 4, "B", 2), ("A", 5, "B", 3), ("A", 6, "A", 7)]

        def grab(jobs, out_ps):
            # free the PSUM accumulators quickly; normalize later from SBUF
            ocs = []
            for i, (j, q) in enumerate(jobs):
                oc = ocp.tile([65, 512], f32, tag="oc", name="oc")
                nc.vector.tensor_copy(oc[:], out_ps[i][:])
                ocs.append(oc)
            return ocs

        def finish_units(jobs, ocs, use_act=False):
            units = []
            cells = [{} for _ in jobs]

            def mk_norm(i, j, q):
                def u():
                    oc = ocs[i]
                    rs = normp.tile([1, 512], f32, tag="rs", name="rs")
                    nc.vector.reciprocal(rs[:], oc[HD:HD + 1, :])
                    rb = normp.tile([HD, 512], f32, tag="rb", name="rb")
                    nc.gpsimd.partition_broadcast(rb[:], rs[:])
                    at = atp.tile([HD, 512], f32r, tag="at", name="at")
                    nc.vector.tensor_mul(at[:], oc[0:HD, :], rb[:])
                    cp = outp.tile([128, KT, 512], f32, tag="cp", name="cp")
                    cells[i]["at"] = at
                    cells[i]["cp"] = cp
                return u

            def mk_proj(i, j, q, m):
                def u():
                    at, cp = cells[i]["at"], cells[i]["cp"]
                    pp = ps_gp.tile([128, 512], f32, tag="gp", name="pp")
                    nc.tensor.matmul(pp[:], wp_t[j][:, m * 128:(m + 1) * 128],
                                     at[:], start=True, stop=True)
                    if use_act and m % 2 == 1:
                        nc.scalar.copy(cp[:, m, :], pp[:])
                    else:
                        nc.vector.tensor_copy(cp[:, m, :], pp[:])
                return u

            def mk_dma(i, j, q):
                def u():
                    ydram = yA if j == "A" else yB
                    ydr = ydram.rearrange("(m p) q -> p m q", p=128)
                    nc.scalar.dma_start(
                        out=ydr[:, :, q * 512:(q + 1) * 512],
                        in_=cells[i]["cp"][:])
                return u

            for i, (j, q) in enumerate(jobs):
                units.append(mk_norm(i, j, q))
                for m in range(KT):
                    units.append(mk_proj(i, j, q, m))
                units.append(mk_dma(i, j, q))
            return units

        hold = {"pending": None}

        def begin_sweep(jobs):
            out_ps = [ps_o.tile([65, 512], f32, tag=f"out{i}", name=f"out{i}")
                      for i in range(2)]
            return {"jobs": jobs, "out": out_ps, "pv": None, "kt": 0}

        def emit_kts(ss, n):
            jobs, out_ps = ss["jobs"], ss["out"]
            for _ in range(n):
                kt = ss["kt"]
                st = ps_st.tile([128, 1024], f32, tag="st", name="st")
                pt = ptp.tile([128, 1024], f32r, tag="pt", name="pt")
                for i, (j, q) in enumerate(jobs):
                    qsrc = QBloc if j == "B" else QAB
                    nc.tensor.matmul(st[:, i * 512:(i + 1) * 512],
                                     KAB[rowsl[j], kt * 128:(kt + 1) * 128],
                                     qsrc[rowsl[j], q * 512:(q + 1) * 512],
                                     start=True, stop=True,
                                     tile_position=tpos[j])
                nc.scalar.activation(pt[:], st[:], EXP)
                if ss["pv"] is not None:
                    ss["pv"]()
                    ss["pv"] = None

                def pv(kt=kt, pt=pt):
                    for i, (j, q) in enumerate(jobs):
                        nc.tensor.matmul(out_ps[i][:],
                                         V_t[:, kt, vo[j]:vo[j] + 65],
                                         pt[:, i * 512:(i + 1) * 512],
                                         start=(kt == 0),
                                         stop=(kt == NKT - 1))
                ss["pv"] = pv
                ss["kt"] += 1
                if ss["kt"] >= 1 and hold["pending"]:
                    hold["pending"].pop(0)()

        def end_sweep(ss, last=False):
            ss["pv"]()
            while hold["pending"]:
                hold["pending"].pop(0)()
            hold["pending"] = finish_units(ss["jobs"],
                                           grab(ss["jobs"], ss["out"]),
                                           use_act=last)

        # fused QKV + sweep 0
        ss0 = begin_sweep([(sweeps[0][0], sweeps[0][1]),
                           (sweeps[0][2], sweeps[0][3])])
        main_chunk(0)
        emit_kts(ss0, 8)
        main_chunk(1)
        emit_kts(ss0, 8)
        main_chunk(2)
        emit_kts(ss0, 8)
        main_chunk(3)
        if SIM_STATIC:
            nc.sync.dma_start(out=QBloc[HD:128, :], in_=QAB[HD:128, 0:SH])
        else:
            with tc.If((pid & 1) < 1) as cmp:
                nc.sync.dma_start(out=QBloc[HD:128, :], in_=QAB[HD:128, 0:SH])
            with cmp.Else():
                nc.sync.dma_start(out=QBloc[HD:128, :],
                                  in_=QAB[HD:128, SH:2 * SH])
        emit_kts(ss0, 8)
        end_sweep(ss0)

        for si, (j0, q0, j1, q1) in enumerate(sweeps[1:]):
            ss = begin_sweep([(j0, q0), (j1, q1)])
            emit_kts(ss, NKT)
            end_sweep(ss, last=(si == len(sweeps) - 2))
        while hold["pending"]:
            hold["pending"].pop(0)()


def _get_nc():
    if "nc" not in _CACHE:
        _CACHE["nc"] = _build_nc()
    return _CACHE["nc"]


def kernel(x, w_qkv, b_qkv, w_proj, b_proj):
    from concourse.bass_utils import run_bass_kernel_spmd

    x = np.asarray(x, dtype=np.float32)
    w_qkv = np.asarray(w_qkv, dtype=np.float32)
    b_qkv = np.asarray(b_qkv, dtype=np.float32)
    w_proj = np.asarray(w_proj, dtype=np.float32)
    b_proj = np.asarray(b_proj, dtype=np.float32)

    B = x.shape[0]
    xT = np.ascontiguousarray(x[0].T)  # [768, 4096]

    def wcol(block, h):
        o = block * DIM + h * HD
        return w_qkv[:, o:o + HD]

    def bcol(block, h):
        o = block * DIM + h * HD
        return b_qkv[o:o + HD]

    in_maps = []
    meta = []
    z64 = np.zeros(HD, dtype=np.float32)
    for c in range(NCORES):
        hA, hB, qh = c, 8 + c // 2, c % 2
        m = {
            "xT": xT,
            "wall": np.concatenate(
                [wcol(0, hA) * SCALE, wcol(1, hB), wcol(1, hA), wcol(2, hB),
                 wcol(2, hA), wcol(0, hB) * SCALE], axis=1),
            "ball": np.stack(
                [np.concatenate([bcol(0, hA) * SCALE, bcol(1, hB)]),
                 np.concatenate([bcol(1, hA), bcol(2, hB)]),
                 np.concatenate([bcol(2, hA), bcol(0, hB) * SCALE]),
                 np.concatenate([z64, z64])], axis=1),
            "wpall": np.concatenate(
                [w_proj[hA * HD:(hA + 1) * HD, :],
                 w_proj[hB * HD:(hB + 1) * HD, :]], axis=1),
        }
        in_maps.append({k: np.ascontiguousarray(v, dtype=np.float32)
                        for k, v in m.items()})
        meta.append(qh)

    nc = _get_nc()
    res = run_bass_kernel_spmd(nc, in_maps, core_ids=list(range(NCORES)))

    Y = np.zeros((DIM, S), dtype=np.float64)
    for c in range(NCORES):
        Y += res.results[c]["yA"].astype(np.float64)
        qh = meta[c]
        Y[:, qh * SH:(qh + 1) * SH] += res.results[c]["yB"].astype(np.float64)
    out = (Y.T + b_proj.astype(np.float64)).astype(np.float32)
    return out.reshape(B, S, DIM)



# revision 34
# speedup vs baseline: 1.1066x; 1.1066x over previous
"""Multi-head self-attention (B=1, S=4096, DIM=768, H=12) on 8 Trainium2
NeuronCores.

Sharding: tensor-parallel over heads. Core c computes
  - full attention for head hA = c            (heads 0..7, all 4096 queries)
  - half attention for head hB = 8 + c//2     (heads 8..11, query half c%2)

Positions are processed in a per-core ROTATED coordinate system
(rot = (true + 2048*qh) mod 4096, qh = c%2). The rotation is applied on the
HOST (each odd core receives np.roll(x^T, -2048, axis=1)) and un-rotated on
the host gather, so the device program is branch-free and identical across
cores. In rotated space head B's queries always occupy columns 0:2048 and
head A's unpaired half 2048:4096.

Six sweeps of (2 jobs x 512 queries) x 32 key-tiles. Sweeps 2-5 pair an
A-slot with the B-slot at the SAME rotated positions: after softmax
normalization the two heads' attention outputs stack on the partition axis
([128,512]) and one K=128 output-projection matmul per 128-feature block
serves both heads (half the proj matmuls and half the y write traffic).
Sweeps 0-1 carry head A's unpaired half; sweep 0 is interleaved with the
QKV projection chunk stream. The last sweep is paired so the final drain is
short.

Engine budget (TRN2 cost model): the Activation engine does ONLY the
softmax exp ([128,1024] per key-tile = 1038ns busy + ~100ns sequencer
overhead) and is the steady-state rate limiter; PE does matmuls
(~215us total, saturated during sweep 0 by the fused QKV stream); DVE does
bias-adds, accumulator grabs and normalization; Pool broadcasts the
reciprocal rows; the idle SP sequencer issues every DMA. Output-projection
results are DMA'd straight from PSUM (no copy engine in the path), dripped
one block per key-tile during the following sweep. Startup loads only the
first weight block + first x half-chunk before compute begins; the tail
ends on a paired sweep with per-block stores.

Matmuls run as float32r (full-rate fp32, ~1e-4 relative error). PV uses a
stationary [V | 1] block so the softmax denominator falls out as row 64 of
the accumulator; no max-subtraction (scores are O(5) for this problem's
N(0,1)-scale inputs, exp cannot overflow fp32).
"""

import numpy as np

DIM = 768
HEADS = 12
HD = 64
SCALE = HD ** (-0.5)
S = 4096
SH = 2048  # seq half
NCORES = 8
KT = DIM // 128  # 6 k-tiles over the 768 contraction dim
NKT = S // 128   # 32 k-tiles over the 4096 sequence dim

# rotated-space chunk start columns, in load order (chunks 0,1 = A's
# unpaired half, 2,3 = the paired half = head B's query range)
RC = [2048, 3072, 0, 1024]
# kt processing order = order key chunks become available
KTORD = [16 + i for i in range(16)] + list(range(16))

_CACHE: dict = {}
SIM_STATIC = False  # kept for test-harness compatibility (program is static)

import os
_DIS = set(os.environ.get("K_DISABLE", "").split(","))  # bisect switches


def _build_nc():
    import concourse.bacc as bacc
    import concourse.tile as tile
    from concourse import mybir
    from concourse.masks import make_identity

    f32 = mybir.dt.float32
    f32r = mybir.dt.float32r
    bf16 = mybir.dt.bfloat16

    nc = bacc.Bacc("TRN2", target_bir_lowering=False)

    # ---- DRAM I/O (per-core) ----
    xT = nc.dram_tensor("xT", [DIM, S], f32r, kind="ExternalInput")
    # packed: [:,0:128]=[wqA*s|wkB], [:,128:256]=[wkA|wvB], [:,256:384]=[wvA|wqB*s]
    wall = nc.dram_tensor("wall", [DIM, 384], f32r, kind="ExternalInput")
    # cols: b1=[bqA*s;bkB], b2=[bkA;bvB], b3=[bvA;bqB*s]
    ball = nc.dram_tensor("ball", [128, 4], f32, kind="ExternalInput")
    # rows 0:64 = w_proj[hA*64:...], rows 64:128 = w_proj[hB*64:...]
    wpall = nc.dram_tensor("wpall", [128, DIM], f32r, kind="ExternalInput")
    # rotated-space partial projection: cols 0:2048 = A+B, 2048:4096 = A only
    yR = nc.dram_tensor("yR", [DIM, S], f32, kind="ExternalOutput")

    with tile.TileContext(nc) as tc:
        _emit(nc, tc, mybir, make_identity, f32, f32r, bf16, xT, wall,
              ball, wpall, yR)

    nc.compile()
    return nc


def _emit(nc, tc, mybir, make_identity, f32, f32r, bf16, xT, wall, ball,
          wpall, yR):
    EXP = mybir.ActivationFunctionType.Exp

    with tc.tile_pool(name="consts", bufs=1) as consts, \
         tc.tile_pool(name="persist", bufs=1) as persist, \
         tc.tile_pool(name="xcp", bufs=2) as xcp, \
         tc.tile_pool(name="vtw", bufs=2) as vtw, \
         tc.tile_pool(name="ptp", bufs=4) as ptp, \
         tc.tile_pool(name="ocp", bufs=4) as ocp, \
         tc.tile_pool(name="normp", bufs=3) as normp, \
         tc.tile_pool(name="atp", bufs=4) as atp, \
         tc.tile_pool(name="outp", bufs=3) as outp, \
         tc.tile_pool(name="ps_gp", bufs=2, space="PSUM") as ps_gp, \
         tc.tile_pool(name="ps_st", bufs=2, space="PSUM") as ps_st, \
         tc.tile_pool(name="ps_o", bufs=1, space="PSUM") as ps_o:

        wallr = wall.rearrange("(k p) c -> p k c", p=128)
        xTr = xT.rearrange("(k p) q -> p k q", p=128)
        yRr = yR.rearrange("(m p) q -> p m q", p=128)

        # ---- startup: constants first (no DMA deps), then DMAs ----
        ident = consts.tile([128, 128], f32, tag="ident", name="ident")
        make_identity(nc, ident[:])
        ones32 = consts.tile([128, 64], f32, tag="ones32", name="ones32")
        nc.vector.memset(ones32[:], 1.0)
        onesr = consts.tile([128, 64], f32r, tag="onesr", name="onesr")
        nc.vector.tensor_copy(onesr[:], ones32[:])

        ballt = consts.tile([128, 4], f32, tag="ball", name="ball")
        nc.sync.dma_start(out=ballt[:], in_=ball[:])
        bias = {f"b{i + 1}": ballt[:, i:i + 1] for i in range(4)}

        wallt = consts.tile([128, KT, 384], f32r, tag="wall", name="wall")
        nc.sync.dma_start(out=wallt[:, :, 0:128], in_=wallr[:, :, 0:128])
        w1_t = [wallt[:, k, 0:128] for k in range(KT)]
        w2_t = [wallt[:, k, 128:256] for k in range(KT)]
        w3_t = [wallt[:, k, 256:384] for k in range(KT)]

        xc0 = xcp.tile([128, KT, 1024], f32r, tag="xc", name="xc")
        nc.sync.dma_start(out=xc0[:, :, 0:512],
                          in_=xTr[:, :, RC[0]:RC[0] + 512])
        nc.sync.dma_start(out=wallt[:, :, 128:256], in_=wallr[:, :, 128:256])
        nc.sync.dma_start(out=wallt[:, :, 256:384], in_=wallr[:, :, 256:384])
        nc.sync.dma_start(out=xc0[:, :, 512:1024],
                          in_=xTr[:, :, RC[0] + 512:RC[0] + 1024])
        wpt = consts.tile([128, DIM], f32r, tag="wpall", name="wpall")
        nc.sync.dma_start(out=wpt[:], in_=wpall[:])

        # PE warm-up: a WAW-chained run of cheap matmuls ending in the ps_gp
        # tiles, so the first QKV matmuls depend on the chain and the
        # scheduler runs it first; keeps the p-state ramp warm while the
        # first x chunk streams in.
        if "warm" not in _DIS:
            for tag in ("wgp1", "wgp2"):
                wg = ps_gp.tile([128, 512], f32, tag="gp", name=tag)
                for i in range(30):
                    nc.tensor.matmul(wg[0:HD, 0:HD], onesr[:], onesr[:],
                                     start=True, stop=True)

        # ---- persistent on-chip tensors (rotated coordinates) ----
        QAB = persist.tile([128, S], f32r, tag="QAB", name="QAB")
        KAB = persist.tile([128, S], f32r, tag="KAB", name="KAB")
        # V_t[:, kt, 0:64]=V_A, col 64=ones, 65:129=V_B, col 129=ones
        V_t = persist.tile([128, NKT, 130], f32r, tag="V", name="V")
        nc.vector.tensor_copy(V_t[:, :, HD], ones32[:, 0:NKT])
        nc.vector.tensor_copy(V_t[:, :, 65 + HD], ones32[:, 0:NKT])

        # ---- QKV projections: x^T streamed once in [128,1024] chunks,
        # processed per 512-col half; xc DMAs issue on the idle Pool SEQ so
        # their buffer-waits never block y-store issues on SP ----
        def load_chunk(j):
            xc = xcp.tile([128, KT, 1024], f32r, tag="xc", name="xc")
            rc = RC[j]
            nc.sync.dma_start(out=xc[:, :, 0:512],
                              in_=xTr[:, :, rc:rc + 512])
            nc.sync.dma_start(out=xc[:, :, 512:1024],
                              in_=xTr[:, :, rc + 512:rc + 1024])
            return xc

        def half_chunk(j, h, xc):
            rc = RC[j]
            cs = slice(rc + h * 512, rc + (h + 1) * 512)
            hs = slice(h * 512, (h + 1) * 512)
            ps1 = ps_gp.tile([128, 512], f32, tag="gp", name="gp1")
            for k in range(KT):
                nc.tensor.matmul(ps1[:], w1_t[k], xc[:, k, hs],
                                 start=(k == 0), stop=(k == KT - 1))
            nc.vector.tensor_scalar_add(QAB[0:HD, cs], ps1[0:HD, :],
                                        bias["b1"][0:HD, :])
            nc.vector.tensor_scalar_add(KAB[HD:128, cs], ps1[HD:128, :],
                                        bias["b1"][HD:128, :])
            ps2 = ps_gp.tile([128, 512], f32, tag="gp", name="gp2")
            for k in range(KT):
                nc.tensor.matmul(ps2[:], w2_t[k], xc[:, k, hs],
                                 start=(k == 0), stop=(k == KT - 1))
            vt = vtw.tile([128, 512], f32, tag="vt", name="vt")
            nc.vector.tensor_scalar_add(KAB[0:HD, cs], ps2[0:HD, :],
                                        bias["b2"][0:HD, :])
            nc.vector.tensor_scalar_add(vt[HD:128, :], ps2[HD:128, :],
                                        bias["b2"][HD:128, :])
            ps3 = ps_gp.tile([128, 512], f32, tag="gp", name="gp3")
            for k in range(KT):
                nc.tensor.matmul(ps3[:], w3_t[k], xc[:, k, hs],
                                 start=(k == 0), stop=(k == KT - 1))
            nc.vector.tensor_scalar_add(vt[0:HD, :], ps3[0:HD, :],
                                        bias["b3"][0:HD, :])
            if j >= 2:  # B's Q only lives at rotated cols 0:2048
                nc.vector.tensor_scalar_add(QAB[HD:128, cs],
                                            ps3[HD:128, :],
                                            bias["b3"][HD:128, :])
            tp = ps_gp.tile([128, 512], f32, tag="gp", name="gp4")
            for t4 in range(4):
                kt = rc // 128 + h * 4 + t4
                ts_ = slice(t4 * 128, (t4 + 1) * 128)
                nc.tensor.transpose(tp[:, ts_], vt[:, ts_], ident[:])
                nc.vector.tensor_copy(V_t[:, kt, 0:HD],
                                      tp[:, t4 * 128:t4 * 128 + HD])
                nc.vector.tensor_copy(V_t[:, kt, 65:65 + HD],
                                      tp[:, t4 * 128 + HD:t4 * 128 + 128])

        # ---- sweeps ----
        rowsl = {"A": slice(0, HD), "B": slice(HD, 128)}
        tpos = {"A": (0, 0), "B": (64, 0)}
        vo = {"A": 0, "B": 65}
        # (job0, slot0, job1, slot1, paired); last sweep paired => short tail
        sweeps = [("A", 4, "A", 5, False), ("A", 6, "A", 7, False),
                  ("A", 0, "B", 0, True), ("A", 1, "B", 1, True),
                  ("A", 2, "B", 2, True), ("A", 3, "B", 3, True)]

        hold = {"pending": None}

        def finish_units(jobs, out_ps, paired, last=False):
            units = []
            cells = {}

            def mk_grab(i):
                def u():
                    oc = ocp.tile([65, 512], f32, tag="oc", name="oc")
                    nc.vector.tensor_copy(oc[:], out_ps[i][:])
                    cells[f"oc{i}"] = oc
                return u

            def mk_norm(i):
                def u():
                    # in the tail (last sweep) read the accumulator straight
                    # from PSUM (no grab in the chain) and broadcast via a
                    # K=1 PE matmul into free PSUM (PE idle, 213ns vs 806)
                    src = out_ps[i] if last else cells[f"oc{i}"]
                    rs = normp.tile([1, 512], f32, tag="rs", name="rs")
                    nc.vector.reciprocal(rs[:], src[HD:HD + 1, :])
                    rb = normp.tile([HD, 512], f32, tag="rb", name="rb")
                    nc.gpsimd.partition_broadcast(rb[:], rs[:])
                    cells[f"rb{i}"] = rb
                return u

            def mk_mul(i):
                def u():
                    if paired:
                        if "at" not in cells:
                            cells["at"] = atp.tile([128, 512], f32r,
                                                   tag="at2", name="at2")
                        at = cells["at"]
                        dst = at[0:HD, :] if i == 0 else at[HD:128, :]
                    else:
                        a = atp.tile([HD, 512], f32r, tag=f"at{i}",
                                     name=f"at{i}")
                        cells[f"at{i}"] = a
                        dst = a[:]
                    nc.vector.tensor_mul(dst, cells[f"oc{i}"][0:HD, :],
                                         cells[f"rb{i}"][:])
                return u

            def mk_proj(i, m, slot):
                def u():
                    at = cells["at" if paired else f"at{i}"]
                    pp = ps_gp.tile([128, 512], f32, tag="gp", name="pp")
                    lhs = wpt[:, m * 128:(m + 1) * 128] if paired \
                        else wpt[0:HD, m * 128:(m + 1) * 128]
                    nc.tensor.matmul(pp[:], lhs, at[:], start=True, stop=True)
                    if "blockdma" in _DIS:
                        ck = f"cpj{i}"
                        if ck not in cells:
                            cells[ck] = outp.tile([128, KT, 512], f32,
                                                  tag="cpj", name="cpj")
                        cp = cells[ck]
                        if last and m % 2 == 1:
                            nc.scalar.copy(cp[:, m, :], pp[:])
                        else:
                            nc.vector.tensor_copy(cp[:, m, :], pp[:])
                        if m == KT - 1:
                            cols = slice(slot * 512, (slot + 1) * 512)
                            nc.sync.dma_start(out=yRr[:, :, cols], in_=cp[:])
                    else:
                        cp = outp.tile([128, 512], f32, tag="cp", name="cp")
                        if last and m % 2 == 1:
                            nc.scalar.copy(cp[:], pp[:])
                        else:
                            nc.vector.tensor_copy(cp[:], pp[:])
                        cols = slice(slot * 512, (slot + 1) * 512)
                        nc.sync.dma_start(out=yRr[:, m, cols], in_=cp[:])
                return u

            if paired:
                if last:
                    # norms read the PSUM accumulators directly; grabs (the
                    # SBUF copies the muls need) overlap the recip chain
                    units.append(mk_norm(0))
                    units.append(mk_norm(1))
                    units.append(mk_grab(0))
                    units.append(mk_grab(1))
                else:
                    units.append(mk_grab(0))
                    units.append(mk_grab(1))
                    units.append(mk_norm(0))
                    units.append(mk_norm(1))
                units.append(mk_mul(0))
                units.append(mk_mul(1))
                for m in range(KT):
                    units.append(mk_proj(0, m, jobs[0][1]))
                    if "space" not in _DIS:
                        units.append(None)
            else:
                for i in range(2):
                    units.append(mk_grab(i))
                    units.append(mk_norm(i))
                    units.append(mk_mul(i))
                    for m in range(KT):
                        units.append(mk_proj(i, m, jobs[i][1]))
                        if i == 1 and "space" not in _DIS:
                            units.append(None)
            return units

        def begin_sweep(jobs):
            out_ps = [ps_o.tile([65, 512], f32, tag=f"out{i}", name=f"out{i}")
                      for i in range(2)]
            return {"jobs": jobs, "out": out_ps, "pv": None, "i": 0}

        def emit_kts(ss, n):
            jobs, out_ps = ss["jobs"], ss["out"]
            for _ in range(n):
                i = ss["i"]
                kt = KTORD[i]
                st = ps_st.tile([128, 1024], f32, tag="st", name="st")
                pt = ptp.tile([128, 1024], f32r, tag="pt", name="pt")
                for jj, (j, q) in enumerate(jobs):
                    nc.tensor.matmul(st[:, jj * 512:(jj + 1) * 512],
                                     KAB[rowsl[j], kt * 128:(kt + 1) * 128],
                                     QAB[rowsl[j], q * 512:(q + 1) * 512],
                                     start=True, stop=True,
                                     tile_position=tpos[j])
                nc.scalar.activation(pt[:], st[:], EXP)
                if ss["pv"] is not None:
                    ss["pv"]()
                    ss["pv"] = None

                def pv(i=i, kt=kt, pt=pt):
                    for jj, (j, q) in enumerate(jobs):
                        nc.tensor.matmul(out_ps[jj][:],
                                         V_t[:, kt, vo[j]:vo[j] + 65],
                                         pt[:, jj * 512:(jj + 1) * 512],
                                         start=(i == 0),
                                         stop=(i == NKT - 1))
                ss["pv"] = pv
                ss["i"] += 1
                dmin = 1 if "space" in _DIS else 2
                if ss["i"] >= dmin and hold["pending"]:
                    u = hold["pending"].pop(0)
                    if u is not None:
                        u()

        def end_sweep(ss, paired, last=False):
            ss["pv"]()
            while hold["pending"]:
                u = hold["pending"].pop(0)
                if u is not None:
                    u()
            hold["pending"] = finish_units(ss["jobs"], ss["out"], paired,
                                           last=last and "tail" not in _DIS)

        # fused QKV + sweep 0 (chunk 0's x halves are already in flight):
        # 4 sweep-0 kts per QKV half-chunk, in key-availability order
        s0 = sweeps[0]
        ss0 = begin_sweep([(s0[0], s0[1]), (s0[2], s0[3])])
        xcs = {0: xc0}
        for j in range(4):
            if j > 0:
                xcs[j] = load_chunk(j)
            # both halves must land before the kt group: the sweep-0 job
            # pair spans the full 1024-col chunk
            half_chunk(j, 0, xcs[j])
            half_chunk(j, 1, xcs[j])
            emit_kts(ss0, 8)
        end_sweep(ss0, s0[4])

        for si, (j0, q0, j1, q1, paired) in enumerate(sweeps[1:]):
            ss = begin_sweep([(j0, q0), (j1, q1)])
            emit_kts(ss, NKT)
            end_sweep(ss, paired, last=(si == len(sweeps) - 2))
        while hold["pending"]:
            u = hold["pending"].pop(0)
            if u is not None:
                u()


def _get_nc():
    if "nc" not in _CACHE:
        _CACHE["nc"] = _build_nc()
    return _CACHE["nc"]


def kernel(x, w_qkv, b_qkv, w_proj, b_proj):
    from concourse.bass_utils import run_bass_kernel_spmd

    x = np.asarray(x, dtype=np.float32)
    w_qkv = np.asarray(w_qkv, dtype=np.float32)
    b_qkv = np.asarray(b_qkv, dtype=np.float32)
    w_proj = np.asarray(w_proj, dtype=np.float32)
    b_proj = np.asarray(b_proj, dtype=np.float32)

    B = x.shape[0]
    xT = np.ascontiguousarray(x[0].T)  # [768, 4096]
    xTrot = np.ascontiguousarray(np.roll(xT, -SH, axis=1))  # odd cores

    def wcol(block, h):
        o = block * DIM + h * HD
        return w_qkv[:, o:o + HD]

    def bcol(block, h):
        o = block * DIM + h * HD
        return b_qkv[o:o + HD]

    in_maps = []
    meta = []
    z64 = np.zeros(HD, dtype=np.float32)
    for c in range(NCORES):
        hA, hB, qh = c, 8 + c // 2, c % 2
        m = {
            "xT": xTrot if qh else xT,
            "wall": np.concatenate(
                [wcol(0, hA) * SCALE, wcol(1, hB), wcol(1, hA), wcol(2, hB),
                 wcol(2, hA), wcol(0, hB) * SCALE], axis=1),
            "ball": np.stack(
                [np.concatenate([bcol(0, hA) * SCALE, bcol(1, hB)]),
                 np.concatenate([bcol(1, hA), bcol(2, hB)]),
                 np.concatenate([bcol(2, hA), bcol(0, hB) * SCALE]),
                 np.concatenate([z64, z64])], axis=1),
            "wpall": np.concatenate(
                [w_proj[hA * HD:(hA + 1) * HD, :],
                 w_proj[hB * HD:(hB + 1) * HD, :]], axis=0),
        }
        in_maps.append({k: np.ascontiguousarray(v, dtype=np.float32)
                        for k, v in m.items()})
        meta.append(qh)

    nc = _get_nc()
    res = run_bass_kernel_spmd(nc, in_maps, core_ids=list(range(NCORES)))

    Y = np.zeros((DIM, S), dtype=np.float64)
    for c in range(NCORES):
        # un-rotate: rotated col rc holds true col (rc + 2048*qh) mod 4096
        Y += np.roll(res.results[c]["yR"].astype(np.float64),
                     meta[c] * SH, axis=1)
    out = (Y.T + b_proj.astype(np.float64)).astype(np.float32)
    return out.reshape(B, S, DIM)
